# revision 8
# baseline (speedup 1.0000x reference)
"""Trainium2 Bass kernel for nn_FAM (dynamic grouped 3x3 low-pass filter + frequency gating).

Data-parallel over batch: 16 images -> 8 cores x 2 images.

Per-core algorithm (all shapes per image):
  pass1: pooled_sum[c] = sum_{h,w} x[c,h,w]          (DVE tensor_scalar accum)
  filt  = tanh(BN(conv_w @ pooled_mean))             (PE matmul + ACT tanh)
  G_dx[h',h] = sum_dy filt[g,dy*3+dx] * delta(h' = reflect(h+dy-1))   (DVE, banded)
  pass2 (per 16-channel half-group, h on partitions, (c,w) on free):
      xp   = col-padded x tile [128, 16*130] (reflect cols)
      xs1  = s1[c] * xp                               (DVE per channel)
      u    = s2[c] * xp + beta[n,c]                   (DVE per channel)
      PSUM = sum_dx G_dx^T @ xs1_dxview + I^T @ u     (PE, fp32r, 4-ch batches)
           = s1*low + s2*x + beta  ==  final output
      out  = ACT copy PSUM -> SBUF -> DMA to DRAM
where s1 = (ia+1)(ll+1)-(lh+1), s2 = lh+1, beta = -ia*(ll+1)*mean(x[c]).
"""

import os
import sys

for _p in ("/opt/trn_rl_repo", "/opt/pypackages"):
    if _p not in sys.path and os.path.isdir(_p):
        sys.path.append(_p)

from contextlib import ExitStack

import numpy as np

import concourse.bass as bass
import concourse.tile as tile
from concourse import bacc
from concourse import mybir
from concourse.bass_utils import run_bass_kernel_spmd

F32 = mybir.dt.float32
F32R = mybir.dt.float32r
AF = mybir.ActivationFunctionType
ALU = mybir.AluOpType

N_CORES = 8
N_PER_CORE = 2        # images per core
C = 256               # channels
G = 8                 # groups
CG = C // G           # 32 channels per group
H = W = 128
HW = H * W
K = 3
BN_EPS = 1e-5
HG_CH = 16            # channels per half-group (shared G weights, 4 matmul batches)
N_HG = C // HG_CH     # 16 half-groups per image
BATCH_CH = 4          # channels per matmul (N = 4*128 = 512)
WPAD = W + 2          # 130: col-padded row length per channel


def _reflect(i: int) -> int:
    if i < 0:
        return -i
    if i > H - 1:
        return 2 * (H - 1) - i
    return i


def _host_consts(conv_w, bn_gamma, bn_beta, bn_mean, bn_var, lamb_l, lamb_h, inside_all):
    """Host-side parameter prep (no x-dependent math)."""
    s_bn = bn_gamma / np.sqrt(bn_var + BN_EPS)
    bn_scale = (s_bn / HW).astype(np.float32)
    bn_bias = (bn_beta - bn_mean * s_bn).astype(np.float32)
    bnsb = np.stack([bn_scale, bn_bias], axis=1)          # [72, 2]

    s1 = (inside_all + 1.0) * (lamb_l + 1.0) - (lamb_h + 1.0)
    s2 = lamb_h + 1.0
    mb = -inside_all * (lamb_l + 1.0) / HW
    sbc = np.concatenate([s1, s2]).astype(np.float32)     # [512]
    sbc = np.broadcast_to(sbc[None, :], (128, 512)).copy()  # [128, 512]
    mbc = mb.astype(np.float32).reshape(2, 128).T.copy()  # [128, 2] c-partition

    d_up = np.zeros((128, 128), np.float32)
    d_dn = np.zeros((128, 128), np.float32)
    idn = np.eye(128, dtype=np.float32)
    for h in range(H):
        d_up[_reflect(h - 1), h] = 1.0
        d_dn[_reflect(h + 1), h] = 1.0
    dmats = np.concatenate([d_up, idn, d_dn], axis=1)     # [128, 384]

    wt = conv_w.T.astype(np.float32)                      # [256, 72]
    wtd = np.concatenate([wt[:128], wt[128:]], axis=1)    # [128, 144]

    return dict(dmats=dmats, sbc=sbc, mbc=mbc, wtd=wtd, bnsb=bnsb)


def _build_kernel(ctx: ExitStack, tc: "tile.TileContext",
                  x_ap: bass.AP, out_ap: bass.AP,
                  dmats_ap: bass.AP, sbc_ap: bass.AP, mbc_ap: bass.AP,
                  wtd_ap: bass.AP, bnsb_ap: bass.AP):
    nc = tc.nc

    cpool = ctx.enter_context(tc.tile_pool(name="consts", bufs=1))
    p1pool = ctx.enter_context(tc.tile_pool(name="pass1", bufs=3))
    stpool = ctx.enter_context(tc.tile_pool(name="stats", bufs=1))
    xppool = ctx.enter_context(tc.tile_pool(name="xp", bufs=3))
    xspool = ctx.enter_context(tc.tile_pool(name="xs1", bufs=2))
    upool = ctx.enter_context(tc.tile_pool(name="u", bufs=2))
    opool = ctx.enter_context(tc.tile_pool(name="outst", bufs=3))
    mpsum = ctx.enter_context(tc.tile_pool(name="mpsum", bufs=6, space="PSUM"))
    spsum = ctx.enter_context(tc.tile_pool(name="spsum", bufs=2, space="PSUM"))

    # ---- constants to SBUF ----
    dmats_sb = cpool.tile([128, 384], F32)
    nc.sync.dma_start(dmats_sb[:], dmats_ap)
    sbc_sb = cpool.tile([128, 512], F32)
    nc.sync.dma_start(sbc_sb[:], sbc_ap)
    mbc_sb = cpool.tile([128, 2], F32)
    nc.sync.dma_start(mbc_sb[:], mbc_ap)
    wtd_sb = cpool.tile([128, 144], F32)
    nc.sync.dma_start(wtd_sb[:], wtd_ap)
    bnsb_sb = cpool.tile([72, 2], F32)
    nc.sync.dma_start(bnsb_sb[:], bnsb_ap)
    ones_sb = cpool.tile([1, 128], F32)
    nc.vector.memset(ones_sb[:], 1.0)

    idn = dmats_sb[:, 128:256]                            # [128,128] identity
    idnr = cpool.tile([128, 128], F32R)
    nc.vector.tensor_copy(idnr[:], idn)

    x_flat = x_ap.rearrange("n c h w -> n c (h w)")       # [2, 256, 16384]

    # persistent per-image tiles
    pooled, fbs, b_n, gt = {}, {}, {}, {}
    for n in range(N_PER_CORE):
        pooled[n] = stpool.tile([128, 2], F32, name=f"pooled_{n}")
        fbs[n] = stpool.tile([128, 72], F32, name=f"fbs_{n}")
        b_n[n] = stpool.tile([128, 256], F32, name=f"bn_{n}")
        gt[n] = stpool.tile([128, G * 3 * 128], F32R, name=f"gt_{n}")

    def pass1(n):
        for b in range(2):
            partials = stpool.tile([128, 8], F32, name=f"partials_{n}_{b}")
            for k in range(8):
                chunk = p1pool.tile([128, 2048], F32, name="chunk")
                nc.sync.dma_start(chunk[:], x_flat[n, bass.ts(b, 128), bass.ts(k, 2048)])
                nc.vector.tensor_scalar(
                    out=chunk[:], in0=chunk[:], scalar1=1.0, scalar2=None,
                    op0=ALU.mult, op1=ALU.add, accum_out=partials[:, k:k + 1])
            nc.vector.tensor_reduce(
                out=pooled[n][:, b:b + 1], in_=partials[:],
                axis=mybir.AxisListType.X, op=ALU.add)

    def filt_branch(n):
        # conv: fpre[j] = sum_c wT[c, j] * pooled_sum[c]
        fpre = spsum.tile([72, 1], F32, name="fpre", tag="sp")
        for b in range(2):
            nc.tensor.matmul(fpre[:], lhsT=wtd_sb[:, b * 72:(b + 1) * 72],
                             rhs=pooled[n][:, b:b + 1],
                             start=(b == 0), stop=(b == 1))
        filt_sb = stpool.tile([72, 1], F32, name=f"filt_{n}")
        nc.scalar.activation(filt_sb[:], fpre[:], AF.Tanh,
                             bias=bnsb_sb[:, 1:2], scale=bnsb_sb[:, 0:1])
        # transpose [72,1] -> [1,72], then broadcast to [128,72]
        ftp = spsum.tile([1, 72], F32, name="ftp", tag="sp")
        nc.tensor.transpose(ftp[:], filt_sb[:], idn[0:72, 0:72])
        filt_row = stpool.tile([1, 72], F32, name=f"filtrow_{n}")
        nc.scalar.copy(filt_row[:], ftp[:])
        fbp = spsum.tile([128, 72], F32, name="fbp", tag="sp")
        nc.tensor.matmul(fbp[:], lhsT=ones_sb[:], rhs=filt_row[:],
                         start=True, stop=True)
        nc.scalar.copy(fbs[n][:], fbp[:])

        # beta row: bcol = pooled_sum * mb (c-partition) -> transpose -> bcast
        bcol = stpool.tile([128, 2], F32, name=f"bcol_{n}")
        nc.vector.tensor_tensor(bcol[:], pooled[n][:], mbc_sb[:], op=ALU.mult)
        for b in range(2):
            btp = spsum.tile([1, 128], F32, name="btp", tag="sp")
            nc.tensor.transpose(btp[:], bcol[:, b:b + 1], idn)
            brow = stpool.tile([1, 128], F32, name=f"brow_{n}_{b}")
            nc.scalar.copy(brow[:], btp[:])
            bbp = spsum.tile([128, 128], F32, name="bbp", tag="sp")
            nc.tensor.matmul(bbp[:], lhsT=ones_sb[:], rhs=brow[:],
                             start=True, stop=True)
            nc.scalar.copy(b_n[n][:, b * 128:(b + 1) * 128], bbp[:])

    def g_build(n):
        # G_dx = f0*D_up + f1*I + f2*D_dn per (g, dx); reflect rows encoded in D mats
        for g in range(G):
            for dx in range(3):
                blk = gt[n][:, (g * 3 + dx) * 128:(g * 3 + dx + 1) * 128]
                j0 = g * 9 + 0 * 3 + dx
                j1 = g * 9 + 1 * 3 + dx
                j2 = g * 9 + 2 * 3 + dx
                nc.vector.tensor_scalar(
                    out=blk, in0=dmats_sb[:, 0:128],
                    scalar1=fbs[n][:, j0:j0 + 1], scalar2=None, op0=ALU.mult)
                nc.vector.scalar_tensor_tensor(
                    out=blk, in0=dmats_sb[:, 128:256],
                    scalar=fbs[n][:, j1:j1 + 1], in1=blk,
                    op0=ALU.mult, op1=ALU.add)
                nc.vector.scalar_tensor_tensor(
                    out=blk, in0=dmats_sb[:, 256:384],
                    scalar=fbs[n][:, j2:j2 + 1], in1=blk,
                    op0=ALU.mult, op1=ALU.add)

    def pass2(n):
        for hg in range(N_HG):
            c0 = hg * HG_CH
            g = c0 // CG
            xp = xppool.tile([128, HG_CH * WPAD], F32, name="xp")
            xp3 = xp.rearrange("p (c w) -> p c w", c=HG_CH)
            # center: DRAM [16,128,128] -> SBUF [128(h), 16(c), 128(w)] at col offset 1
            nc.sync.dma_start(xp3[:, :, 1:129],
                              x_ap[n, c0:c0 + HG_CH, :, :].transpose([1, 0, 2]))
            # reflect cols: xp[...,0] = x[...,1] (xp col 2); xp[...,129] = x[...,126] (xp col 127)
            nc.vector.tensor_copy(xp3[:, :, 0:1], xp3[:, :, 2:3])
            nc.vector.tensor_copy(xp3[:, :, 129:130], xp3[:, :, 127:128])

            xs1 = xspool.tile([128, HG_CH * WPAD], F32R, name="xs1")
            xs13 = xs1.rearrange("p (c w) -> p c w", c=HG_CH)
            u = upool.tile([128, HG_CH * WPAD], F32R, name="u")
            u3 = u.rearrange("p (c w) -> p c w", c=HG_CH)
            for cc in range(HG_CH):
                c = c0 + cc
                nc.vector.tensor_scalar(
                    out=xs13[:, cc, :], in0=xp3[:, cc, :],
                    scalar1=sbc_sb[:, c:c + 1], scalar2=None, op0=ALU.mult)
                nc.vector.tensor_scalar(
                    out=u3[:, cc, :], in0=xp3[:, cc, :],
                    scalar1=sbc_sb[:, 256 + c:256 + c + 1],
                    scalar2=b_n[n][:, c:c + 1],
                    op0=ALU.mult, op1=ALU.add)

            outst = opool.tile([128, HG_CH * W], F32, name="outst")
            outst3 = outst.rearrange("p (c w) -> p c w", c=HG_CH)
            ps = [mpsum.tile([128, 512], F32, name="ps", tag="ps")
                  for _ in range(HG_CH // BATCH_CH)]
            w_aps = [gt[n][:, (g * 3 + dx) * 128:(g * 3 + dx + 1) * 128]
                     for dx in range(3)]
            w_aps.append(idnr[:])
            for wi in range(4):
                for q in range(HG_CH // BATCH_CH):
                    if wi < 3:
                        rhs = xs13[:, q * BATCH_CH:(q + 1) * BATCH_CH, wi:wi + 128]
                    else:
                        rhs = u3[:, q * BATCH_CH:(q + 1) * BATCH_CH, 1:129]
                    nc.tensor.matmul(ps[q][:], lhsT=w_aps[wi], rhs=rhs,
                                     start=(wi == 0), stop=(wi == 3))
            for q in range(HG_CH // BATCH_CH):
                nc.scalar.copy(
                    outst3[:, q * BATCH_CH:(q + 1) * BATCH_CH, :], ps[q][:])
            nc.sync.dma_start(out_ap[n, c0:c0 + HG_CH, :, :].transpose([1, 0, 2]),
                              outst3[:, :, :])

    pass1(0)
    filt_branch(0)
    g_build(0)
    pass1(1)
    pass2(0)
    filt_branch(1)
    g_build(1)
    pass2(1)


def build_nc():
    nc = bacc.Bacc("TRN2", target_bir_lowering=False, debug=False)
    x_h = nc.dram_tensor("x", [N_PER_CORE, C, H, W], F32, kind="ExternalInput")
    dmats_h = nc.dram_tensor("dmats", [128, 384], F32, kind="ExternalInput")
    sbc_h = nc.dram_tensor("sbc", [128, 512], F32, kind="ExternalInput")
    mbc_h = nc.dram_tensor("mbc", [128, 2], F32, kind="ExternalInput")
    wtd_h = nc.dram_tensor("wtd", [128, 144], F32, kind="ExternalInput")
    bnsb_h = nc.dram_tensor("bnsb", [72, 2], F32, kind="ExternalInput")
    out_h = nc.dram_tensor("out", [N_PER_CORE, C, H, W], F32, kind="ExternalOutput")

    with tile.TileContext(nc) as tc:
        with ExitStack() as ctx:
            _build_kernel(ctx, tc, x_h.ap(), out_h.ap(), dmats_h.ap(),
                          sbc_h.ap(), mbc_h.ap(), wtd_h.ap(), bnsb_h.ap())
    nc.compile()
    return nc


def kernel(x, conv_w, bn_gamma, bn_beta, bn_mean, bn_var, lamb_l, lamb_h,
           inside_all, _trace=False, _trace_kwargs=None):
    x = np.ascontiguousarray(x, dtype=np.float32)
    consts = _host_consts(conv_w, bn_gamma, bn_beta, bn_mean, bn_var,
                          lamb_l, lamb_h, inside_all)
    nc = build_nc()
    in_maps = []
    for i in range(N_CORES):
        m = {"x": x[i * N_PER_CORE:(i + 1) * N_PER_CORE]}
        m.update(consts)
        in_maps.append(m)
    kw = {}
    if _trace:
        kw["trace"] = True
        if _trace_kwargs:
            kw.update(_trace_kwargs)
    res = run_bass_kernel_spmd(nc, in_maps, list(range(N_CORES)), **kw)
    out = np.concatenate([res.results[i]["out"] for i in range(N_CORES)], axis=0)
    if _trace:
        kernel.last_results = res
    return out
